# revision 23
# baseline (speedup 1.0000x reference)
"""DecoderRNN (GRU + embedding + vocab projection) Bass kernel for 8 trn2 cores.

Model (per reference):
  toks = [2, x[0..S-2]]                          (teacher forcing, S=64, B=64)
  e_s  = relu(emb[toks_s])                       (E=512, padding row 0 = 0)
  GRU: r = sig(e@Wir^T + b_ir + h@Whr^T + b_hr)
       z = sig(e@Wiz^T + b_iz + h@Whz^T + b_hz)
       n = tanh(e@Win^T + b_in + r*(h@Whn^T + b_hn))
       h' = (1-z)*n + z*h                        (H=1024)
  logits_s = h_s @ Wout^T + b_out                (V=32000)
  out = logits.transpose(1,0,2)[None]            -> (1, B, S, V) f32

Distribution: the GRU recurrence is replicated on all 8 cores; the output
projection is vocab-split 8 ways (4000 cols per core), its matmuls
statically interleaved into the recurrence.

v2 vs v1 (same fp16 math — fp8/DoubleRow was measured a net loss here:
DR forbids the column-tiled psum quadrants this layout relies on):
 - recurrence matmuls ordered r-pieces -> hn-pieces -> z-pieces so the
   r-sigmoid / tanh chain starts ~2us earlier each step (the z gate is
   only needed at the end of the chain);
 - the gi-add is split r/z to match;
 - SBUF-only gate-chain ops (tn-add, h' chain, fp16 cast) run on the
   gpsimd (Pool) engine, leaving the DVE to the PSUM-touching ops;
 - output jobs emitted as nn-pairs sharing one [128,1024] psum tile and
   one epilogue tensor_tensor; output DMA moved to the sync ring.

Layouts ("stacked" = batch folded into 128 partitions as two 512-wide
halves; partition p <-> (half=p//64, b=p%64)):
  psum_rz [128,1024]: cols j<512 -> gate r col 512*half+j, j>=512 -> z
  psum_hn [128,512]: col j -> n-gate h-part col 512*half+j
  giS [128,1536] fp16: cols [r|z|n] per-half slices of gi
  hT [128, 8*64] fp16: col 64k+b, partition p -> h[b, 128k+p]  (matmul lhsT)
Matmuls run fp16 (exact products, fp32 PSUM accumulate); gate arithmetic
fp32; state h kept fp32 (stacked) + fp16 (transposed, for lhsT); logits
stored fp16.  All transposes are PE-transposes (identity matmul).
"""

import sys

sys.path.insert(0, "/opt/trn_rl_repo")

import numpy as np

import concourse.bass as bass
import concourse.bacc as bacc
import concourse.mybir as mybir
import concourse.tile as tile
from concourse.bass_utils import run_bass_kernel_spmd
from concourse.masks import make_identity

FP16 = mybir.dt.float16
F32 = mybir.dt.float32
I32 = mybir.dt.int32

S, B, H, E, V = 64, 64, 1024, 512, 32000
NCORES = 8
VC = V // NCORES          # 4000 vocab cols per core
SB = S * B                # 4096
KH = H // 128             # 8 h k-chunks
KE = E // 128             # 4 e k-chunks
NQ = 4                    # output job-pairs per 128-token tile
NT = SB // 128            # 32 token tiles (also output row tiles)
G3 = 3 * H                # 3072 gate cols
GLEAD = 4                 # gi tiles emitted ahead of the step loop

_CACHE = {}


def _build(n_steps=S, with_jobs=True):
    key = ("nc", n_steps, with_jobs)
    if key in _CACHE:
        return _CACHE[key]

    nc = bacc.Bacc("TRN2", target_bir_lowering=False, debug=False)

    def din(name, shape, dt):
        return nc.dram_tensor(name, shape, dt, kind="ExternalInput").ap()

    emb_d = din("emb_t", [V, E], FP16)
    idx_d = din("idx", [128, NT], I32)
    w_rzA_d = din("w_rzA", [128, KH, 1024], FP16)
    w_rzB_d = din("w_rzB", [128, KH, 1024], FP16)
    w_hnA_d = din("w_hnA", [128, KH, 512], FP16)
    w_hnB_d = din("w_hnB", [128, KH, 512], FP16)
    w_iflat_d = din("w_iflat", [128, KE, G3], FP16)
    gi_bias_d = din("gi_bias", [128, G3], FP16)
    bias_nhh_d = din("bias_nhh", [128, 512], F32)
    hT0_d = din("hT0", [128, 4, 128], FP16)
    h0st_d = din("h0st", [128, 512], F32)
    w_outT_d = din("w_outT", [128, KH, VC], FP16)
    b_out_d = din("b_out_pair", [128, NQ, 1024], FP16)
    out_d = nc.dram_tensor("out", [SB, VC], FP16, kind="ExternalOutput").ap()

    # per-tile DRAM staging for gi (one tensor per 128-token tile so tile
    # dep tracking, whole-tensor granular on DRAM, stays exact)
    gi_d = [nc.dram_tensor(f"gi{j}", [128, G3], FP16).ap() for j in range(NT)]

    with tile.TileContext(nc) as tc:
        with tc.tile_pool(name="const", bufs=1) as pc, \
             tc.tile_pool(name="roll", bufs=1) as pr, \
             tc.tile_pool(name="psum", bufs=1, space="PSUM") as pp:

            # ---- constants in SBUF
            w_rzA = pc.tile([128, KH, 1024], FP16, name="w_rzA")
            w_rzB = pc.tile([128, KH, 1024], FP16, name="w_rzB")
            w_hnA = pc.tile([128, KH, 512], FP16, name="w_hnA")
            w_hnB = pc.tile([128, KH, 512], FP16, name="w_hnB")
            w_iflat = pc.tile([128, KE, G3], FP16, name="w_iflat")
            gi_bias = pc.tile([128, G3], FP16, name="gi_bias")
            bias_nhh = pc.tile([128, 512], F32, name="bias_nhh")
            w_outT = pc.tile([128, KH, VC], FP16, name="w_outT")
            b_out = pc.tile([128, NQ, 1024], FP16, name="b_out")
            idx = pc.tile([128, NT], I32, name="idx")
            ident = pc.tile([128, 128], FP16, name="ident")

            # DMA order: idx first (gates the gathers), then w_iflat (gates
            # the gi precompute that fills the startup window), then the
            # recurrence weights, output consts last.
            nc.sync.dma_start(out=idx[:], in_=idx_d[:])
            make_identity(nc, ident[:])

            hT = pr.tile([128, 4, 128], FP16, name="hT_init", tag="hT", bufs=2)
            h_st = pr.tile([128, 512], F32, name="hst_init", tag="hst", bufs=2)
            nc.sync.dma_start(out=hT[:], in_=hT0_d[:])
            nc.sync.dma_start(out=h_st[:], in_=h0st_d[:])

            for t, d in [(w_iflat, w_iflat_d), (gi_bias, gi_bias_d),
                         (bias_nhh, bias_nhh_d),
                         (w_rzA, w_rzA_d), (w_rzB, w_rzB_d),
                         (w_hnA, w_hnA_d), (w_hnB, w_hnB_d),
                         (b_out, b_out_d), (w_outT, w_outT_d)]:
                nc.sync.dma_start(out=t[:], in_=d[:])

            # ---- HAM warmup: dummy PE transposes spanning the initial
            # weight-DMA window.  The PE clock gate needs ~3.4us of
            # sustained busy to reach 2.4 GHz and re-throttles after any
            # ~3.4us idle; without these the whole gi/recurrence startup
            # runs at 1.2 GHz.
            psd = pp.tile([128, 128], FP16, name="ps_warm", tag="psT",
                          bufs=1)
            for _ in range(256):
                nc.tensor.matmul(out=psd[:], lhsT=ident[:], rhs=ident[:],
                                 is_transpose=True, start=True, stop=True,
                                 skip_group_check=True)

            # ---- batched gi precompute, one 128-token tile at a time
            def emit_gi(j):
                er = pr.tile([128, E], FP16, name=f"er{j}", tag="er", bufs=2)
                nc.gpsimd.indirect_dma_start(
                    out=er[:], out_offset=None,
                    in_=emb_d[:],
                    in_offset=bass.IndirectOffsetOnAxis(
                        ap=idx[:, j:j + 1], axis=0),
                )
                ps_e = pp.tile([128, 512], FP16, name=f"psE{j}", tag="psT",
                               bufs=1)
                for m in range(KE):
                    nc.tensor.matmul(
                        out=ps_e[:, 128 * m:128 * m + 128],
                        lhsT=er[:, 128 * m:128 * m + 128],
                        rhs=ident[:],
                        is_transpose=True, start=(m == 0), stop=(m == KE - 1),
                        skip_group_check=True)
                eT = pr.tile([128, KE, 128], FP16, name=f"eT{j}", tag="eT",
                             bufs=2)
                nc.vector.tensor_copy(
                    out=eT[:], in_=ps_e[:].rearrange("p (m c) -> p m c", m=KE))
                gi16 = pr.tile([128, G3], FP16, name=f"gi16_{j}", tag="gi16",
                               bufs=2)
                for v in range(6):
                    ps = pp.tile([128, 512], F32, name=f"psg{j}_{v}",
                                 tag="psgi", bufs=2)
                    for k in range(KE):
                        nc.tensor.matmul(
                            out=ps[:], lhsT=eT[:, k, :],
                            rhs=w_iflat[:, k, 512 * v:512 * v + 512],
                            start=(k == 0), stop=(k == KE - 1),
                            skip_group_check=True)
                    nc.vector.tensor_tensor(
                        out=gi16[:, 512 * v:512 * v + 512], in0=ps[:],
                        in1=gi_bias[:, 512 * v:512 * v + 512],
                        op=mybir.AluOpType.add)
                # sync ring (NOT scalar: there it waited ~16us per tile at
                # the ring head, blocking the gate-chain activations queued
                # behind it)
                nc.sync.dma_start(out=gi_d[j][:], in_=gi16[:])

            for j in range(min(GLEAD, NT)):
                emit_gi(j)

            # ---- per-step gi slices in stacked layout, prefetched
            def fetch_gi(s):
                t = pr.tile([128, 3 * 512], FP16, name=f"giS{s}", tag="giS",
                            bufs=3)
                view = gi_d[s // 2][64 * (s % 2):64 * (s % 2) + 64,
                                    :].rearrange(
                    "b (g hf j) -> hf b g j", g=3, hf=2, j=512)
                for hf in (0, 1):
                    # sync ring (free after const loads) — keeps the reads
                    # out of the scalar ring where the gi_d writes queue
                    nc.sync.dma_start(
                        out=t[64 * hf:64 * hf + 64, :].rearrange(
                            "p (g j) -> p g j", g=3, j=512),
                        in_=view[hf])
                return t

            gi_w = {s: fetch_gi(s) for s in range(min(3, n_steps))}

            hs_w = {}     # output-ready hidden tiles: t -> [128, KH, 128] fp16

            jobs = [(t, q) for t in range(n_steps // 2) for q in range(NQ)]
            if not with_jobs:
                jobs = []
            jp = 0  # job pointer

            def emit_job(t, q):
                ps_o = pp.tile([128, 1024], F32, name=f"pso{t}_{q}",
                               tag="pso", bufs=1)
                hst_t = hs_w[t]
                for k in range(KH):
                    lh = hst_t[:, k, :]
                    st, sp = (k == 0), (k == KH - 1)
                    nc.tensor.matmul(
                        out=ps_o[:, 0:500], lhsT=lh,
                        rhs=w_outT[:, k, 1000 * q:1000 * q + 500],
                        start=st, stop=sp, skip_group_check=True)
                    nc.tensor.matmul(
                        out=ps_o[:, 512:1012], lhsT=lh,
                        rhs=w_outT[:, k, 1000 * q + 500:1000 * q + 1000],
                        start=st, stop=sp, skip_group_check=True)
                ob = pr.tile([128, 1024], FP16, name=f"ob{t}_{q}", tag="ob",
                             bufs=2)
                nc.vector.tensor_tensor(
                    out=ob[:, 0:1012], in0=ps_o[:, 0:1012],
                    in1=b_out[:, q, 0:1012], op=mybir.AluOpType.add)
                nc.sync.dma_start(
                    out=out_d[t * 128:(t + 1) * 128,
                              1000 * q:1000 * q + 500],
                    in_=ob[:, 0:500])
                nc.sync.dma_start(
                    out=out_d[t * 128:(t + 1) * 128,
                              1000 * q + 500:1000 * q + 1000],
                    in_=ob[:, 512:1012])

            for s in range(n_steps):
                g, half = s // 2, s % 2

                # ---- recurrence h-part matmuls (lanes A/B = psum halves).
                # Piece order r -> hn -> z: the r/tanh chain is the longest
                # serial path, the z gate is only needed at the end.
                ps_rz = pp.tile([128, 1024], F32, name=f"psrz{s}", tag="psrz",
                                bufs=1)
                ps_hn = pp.tile([128, 512], F32, name=f"pshn{s}", tag="pshn",
                                bufs=1)

                def lhk(k):
                    return hT[:, k % 4, 64 * (k // 4):64 * (k // 4) + 64]

                for k in range(KH):
                    lh, st, sp = lhk(k), (k == 0), (k == KH - 1)
                    nc.tensor.matmul(out=ps_rz[0:64, 0:512], lhsT=lh,
                                     rhs=w_rzA[:, k, 0:512], start=st, stop=sp,
                                     skip_group_check=True)
                    nc.tensor.matmul(out=ps_rz[64:128, 0:512], lhsT=lh,
                                     rhs=w_rzB[:, k, 0:512], start=st, stop=sp,
                                     skip_group_check=True)
                for k in range(KH):
                    lh, st, sp = lhk(k), (k == 0), (k == KH - 1)
                    nc.tensor.matmul(out=ps_hn[0:64, :], lhsT=lh,
                                     rhs=w_hnA[:, k, :],
                                     start=st, stop=sp, skip_group_check=True)
                    nc.tensor.matmul(out=ps_hn[64:128, :], lhsT=lh,
                                     rhs=w_hnB[:, k, :],
                                     start=st, stop=sp, skip_group_check=True)
                for k in range(KH):
                    lh, st, sp = lhk(k), (k == 0), (k == KH - 1)
                    nc.tensor.matmul(out=ps_rz[0:64, 512:1024], lhsT=lh,
                                     rhs=w_rzA[:, k, 512:1024], start=st,
                                     stop=sp, skip_group_check=True)
                    nc.tensor.matmul(out=ps_rz[64:128, 512:1024], lhsT=lh,
                                     rhs=w_rzB[:, k, 512:1024], start=st,
                                     stop=sp, skip_group_check=True)

                # ---- prefetch gi slice for a later step.  Emitted FIRST so
                # it lands on the sync ring ahead of this step's output DMAs
                # (whose data isn't ready yet — head-of-line blocking here
                # starves the recurrence of gi and stalls the PE).
                if s + 3 < n_steps:
                    gi_w[s + 3] = fetch_gi(s + 3)

                # ---- gate chain.  DVE keeps the PSUM-touching ops (Pool
                # cannot read PSUM); gpsimd takes the SBUF-only tail.
                giS = gi_w.pop(s)
                rz = pr.tile([128, 1024], FP16, name=f"rz{s}", tag="rz",
                             bufs=2)
                nc.vector.tensor_tensor(out=ps_rz[:, 0:512],
                                        in0=ps_rz[:, 0:512],
                                        in1=giS[:, 0:512],
                                        op=mybir.AluOpType.add)
                nc.scalar.activation(out=rz[:, 0:512], in_=ps_rz[:, 0:512],
                                     func=mybir.ActivationFunctionType.Sigmoid)
                nc.vector.tensor_tensor(out=ps_hn[:], in0=ps_hn[:],
                                        in1=bias_nhh[:],
                                        op=mybir.AluOpType.add)
                tn = pr.tile([128, 512], FP16, name=f"tn{s}", tag="tn",
                             bufs=2)
                nc.vector.tensor_tensor(out=tn[:], in0=rz[:, 0:512],
                                        in1=ps_hn[:],
                                        op=mybir.AluOpType.mult)
                nc.vector.tensor_tensor(out=ps_rz[:, 512:1024],
                                        in0=ps_rz[:, 512:1024],
                                        in1=giS[:, 512:1024],
                                        op=mybir.AluOpType.add)
                nc.gpsimd.tensor_tensor(out=tn[:], in0=tn[:],
                                        in1=giS[:, 1024:1536],
                                        op=mybir.AluOpType.add)
                n_sb = pr.tile([128, 512], F32, name=f"n{s}", tag="n", bufs=1)
                nc.scalar.activation(out=n_sb[:], in_=tn[:],
                                     func=mybir.ActivationFunctionType.Tanh)
                nc.scalar.activation(out=rz[:, 512:1024],
                                     in_=ps_rz[:, 512:1024],
                                     func=mybir.ActivationFunctionType.Sigmoid)
                d_sb = pr.tile([128, 512], F32, name=f"d{s}", tag="d", bufs=1)
                nc.gpsimd.tensor_tensor(out=d_sb[:], in0=h_st[:], in1=n_sb[:],
                                        op=mybir.AluOpType.subtract)
                nc.gpsimd.tensor_tensor(out=d_sb[:], in0=rz[:, 512:1024],
                                        in1=d_sb[:],
                                        op=mybir.AluOpType.mult)
                h_st = pr.tile([128, 512], F32, name=f"hst{s}", tag="hst",
                               bufs=2)
                nc.gpsimd.tensor_tensor(out=h_st[:], in0=n_sb[:], in1=d_sb[:],
                                        op=mybir.AluOpType.add)

                # ---- PE filler sized to the gate-chain latency window: gi
                # tile (phase 1) or two job-pairs (phase 2) run while the
                # chain computes h'; then the transpose; then the rest of
                # the jobs overlap the NEXT step's chain.
                nj = 2 if s + GLEAD < NT else 3
                njb = 0 if s + GLEAD < NT else 2
                emitted = 0
                if s + GLEAD < NT:
                    emit_gi(s + GLEAD)
                if s >= 3:
                    while (emitted < njb and jp < len(jobs)
                           and 2 * jobs[jp][0] + 2 <= s):
                        emit_job(*jobs[jp])
                        jp += 1
                        emitted += 1

                # ---- transpose h back to lhsT layout.  The hT copy runs on
                # the ACT engine (free after the sigmoids) so it never queues
                # behind epilogue adds on the DVE — it gates the next step's
                # recurrence matmuls.
                h16 = pr.tile([128, 512], FP16, name=f"h16_{s}", tag="h16",
                              bufs=2)
                nc.gpsimd.tensor_copy(out=h16[:], in_=h_st[:])
                ps_T = pp.tile([128, 512], FP16, name=f"psT{s}", tag="psT",
                               bufs=1)
                for m in range(4):
                    nc.tensor.matmul(
                        out=ps_T[:, 128 * m:128 * m + 128],
                        lhsT=h16[:, 128 * m:128 * m + 128],
                        rhs=ident[:],
                        is_transpose=True, start=(m == 0), stop=(m == 3),
                        skip_group_check=True)
                hT = pr.tile([128, 4, 128], FP16, name=f"hT{s}", tag="hT",
                             bufs=2)
                nc.scalar.copy(
                    out=hT[:], in_=ps_T[:].rearrange("p (m c) -> p m c", m=4))
                if half == 0:
                    hs_w[g] = pr.tile([128, KH, 128], FP16, name=f"hs{g}",
                                      tag="hs", bufs=3)
                nc.vector.tensor_copy(
                    out=hs_w[g][:, :, 64 * half:64 * half + 64],
                    in_=ps_T[:].rearrange("p (m hh b) -> p hh m b", m=4, hh=2))

                # ---- remaining jobs overlap the next step's gate chain
                if s >= 3:
                    while (emitted < nj and jp < len(jobs)
                           and 2 * jobs[jp][0] + 2 <= s):
                        emit_job(*jobs[jp])
                        jp += 1
                        emitted += 1

            # ---- drain remaining output jobs
            while jp < len(jobs):
                emit_job(*jobs[jp])
                jp += 1

    nc.compile()
    _CACHE[key] = nc
    return nc


def _prep_in_maps(x, hidden, emb, w_ih, w_hh, b_ih, b_hh, w_out, b_out):
    f16, f32 = np.float16, np.float32

    toks = np.concatenate([np.full((1, B), 2, dtype=np.int64),
                           np.asarray(x)[:-1].astype(np.int64)], axis=0)
    t_flat = toks.reshape(SB).astype(np.int32)
    idx = np.ascontiguousarray(t_flat.reshape(NT, 128).T)        # [128, 32]

    emb_t = np.asarray(emb, dtype=f32).copy()
    emb_t[0] = 0.0
    emb_t = np.maximum(emb_t, 0.0).astype(f16)                    # relu folded

    w_hh = np.asarray(w_hh, dtype=f32)
    w_ih = np.asarray(w_ih, dtype=f32)
    Wr, Wz, Wn = w_hh[0:H], w_hh[H:2 * H], w_hh[2 * H:3 * H]

    def kview(m, kc):  # [rows, K] -> [128, kc, rows] fp16 (K on partitions)
        return np.ascontiguousarray(
            m.T.reshape(kc, 128, m.shape[0]).transpose(1, 0, 2)).astype(f16)

    w_rzA = kview(np.concatenate([Wr[0:512], Wz[0:512]], 0), KH)
    w_rzB = kview(np.concatenate([Wr[512:1024], Wz[512:1024]], 0), KH)
    w_hnA = kview(Wn[0:512], KH)
    w_hnB = kview(Wn[512:1024], KH)
    w_iflat = kview(w_ih, KE)                                     # [128,4,3072]

    b_ih = np.asarray(b_ih, dtype=f32)
    b_hh = np.asarray(b_hh, dtype=f32)
    gb = np.concatenate([b_ih[0:2 * H] + b_hh[0:2 * H], b_ih[2 * H:3 * H]])
    gi_bias = np.ascontiguousarray(
        np.broadcast_to(gb, (128, G3))).astype(f16)
    bias_nhh = np.empty((128, 512), f32)
    for hp in (0, 1):
        r = slice(64 * hp, 64 * hp + 64)
        bias_nhh[r] = b_hh[2 * H:3 * H][512 * hp:512 * hp + 512][None, :]

    h0 = np.asarray(hidden, dtype=f32)[0]                         # [B, H]
    # hT0[p, m, 64*hh + b] = h0[b, 128*(m + 4*hh) + p]
    hT0 = np.ascontiguousarray(
        h0.T.reshape(2, 4, 128, B).transpose(2, 1, 0, 3).reshape(128, 4, 128)
    ).astype(f16)
    h0st = np.concatenate([h0[:, 0:512], h0[:, 512:1024]], axis=0).astype(f32)

    w_out = np.asarray(w_out, dtype=f32)
    b_out = np.asarray(b_out, dtype=f32)

    shared = dict(
        emb_t=emb_t, idx=idx,
        w_rzA=w_rzA, w_rzB=w_rzB, w_hnA=w_hnA, w_hnB=w_hnB,
        w_iflat=w_iflat, gi_bias=gi_bias, bias_nhh=bias_nhh,
        hT0=hT0, h0st=h0st,
    )
    in_maps = []
    for c in range(NCORES):
        sl = slice(c * VC, (c + 1) * VC)
        w_outT = np.ascontiguousarray(
            w_out[sl].T.reshape(KH, 128, VC).transpose(1, 0, 2)).astype(f16)
        bo = b_out[sl]
        b_out_pair = np.zeros((128, NQ, 1024), f16)
        for q in range(NQ):
            b_out_pair[:, q, 0:500] = bo[1000 * q:1000 * q + 500][None]
            b_out_pair[:, q, 512:1012] = bo[1000 * q + 500:1000 * q + 1000][None]
        in_maps.append(dict(shared, w_outT=w_outT, b_out_pair=b_out_pair))
    return in_maps


def _assemble(results):
    full = np.concatenate(
        [r["out"].reshape(S, B, VC) for r in results], axis=2)   # (S, B, V)
    return np.ascontiguousarray(full.transpose(1, 0, 2)[None]).astype(np.float32)


def _run(trace=False, tmpdir=None, **inputs):
    nc = _build()
    in_maps = _prep_in_maps(**inputs)
    res = run_bass_kernel_spmd(nc, in_maps, list(range(NCORES)),
                               trace=trace, tmpdir=tmpdir)
    return _assemble(res.results), res


def kernel(**inputs) -> np.ndarray:
    out, _ = _run(**inputs)
    return out


if __name__ == "__main__":
    rng = np.random.default_rng(0)
    ins = dict(
        x=rng.integers(0, V, (S, B)).astype(np.int32),
        hidden=rng.standard_normal((1, B, H)).astype(np.float32),
        emb=rng.standard_normal((V, E)).astype(np.float32),
        w_ih=rng.uniform(-1 / 32, 1 / 32, (3 * H, E)).astype(np.float32),
        w_hh=rng.uniform(-1 / 32, 1 / 32, (3 * H, H)).astype(np.float32),
        b_ih=rng.uniform(-1 / 32, 1 / 32, (3 * H,)).astype(np.float32),
        b_hh=rng.uniform(-1 / 32, 1 / 32, (3 * H,)).astype(np.float32),
        w_out=rng.uniform(-1 / 32, 1 / 32, (V, H)).astype(np.float32),
        b_out=rng.uniform(-1 / 32, 1 / 32, (V,)).astype(np.float32),
    )
    out = kernel(**ins)
    print("out", out.shape, out.dtype, float(np.abs(out).max()))


# revision 28
# speedup vs baseline: 1.1896x; 1.1896x over previous
"""DecoderRNN (GRU + embedding + vocab projection) Bass kernel for 8 trn2 cores.

Model (per reference):
  toks = [2, x[0..S-2]]                          (teacher forcing, S=64, B=64)
  e_s  = relu(emb[toks_s])                       (E=512, padding row 0 = 0)
  GRU: r = sig(e@Wir^T + b_ir + h@Whr^T + b_hr)
       z = sig(e@Wiz^T + b_iz + h@Whz^T + b_hz)
       n = tanh(e@Win^T + b_in + r*(h@Whn^T + b_hn))
       h' = (1-z)*n + z*h                        (H=1024)
  logits_s = h_s @ Wout^T + b_out                (V=32000)
  out = logits.transpose(1,0,2)[None]            -> (1, B, S, V) f32

Distribution: the GRU recurrence is replicated on all 8 cores; the output
projection is vocab-split 8 ways (4000 cols per core), its matmuls
statically interleaved into the recurrence.

v2 vs v1 (same fp16 math — fp8/DoubleRow was measured a net loss here:
DR forbids the column-tiled psum quadrants this layout relies on):
 - recurrence matmuls ordered r-pieces -> hn-pieces -> z-pieces so the
   r-sigmoid / tanh chain starts ~2us earlier each step (the z gate is
   only needed at the end of the chain);
 - the gi-add is split r/z to match;
 - SBUF-only gate-chain ops (tn-add, h' chain, fp16 cast) run on the
   gpsimd (Pool) engine, leaving the DVE to the PSUM-touching ops;
 - output jobs emitted as nn-pairs sharing one [128,1024] psum tile and
   one epilogue tensor_tensor; output DMA moved to the sync ring.

Layouts ("stacked" = batch folded into 128 partitions as two 512-wide
halves; partition p <-> (half=p//64, b=p%64)):
  psum_rz [128,1024]: cols j<512 -> gate r col 512*half+j, j>=512 -> z
  psum_hn [128,512]: col j -> n-gate h-part col 512*half+j
  giS [128,1536] fp16: cols [r|z|n] per-half slices of gi
  hT [128, 8*64] fp16: col 64k+b, partition p -> h[b, 128k+p]  (matmul lhsT)
Matmuls run fp16 (exact products, fp32 PSUM accumulate); gate arithmetic
fp32; state h kept fp32 (stacked) + fp16 (transposed, for lhsT); logits
stored fp16.  All transposes are PE-transposes (identity matmul).
"""

import sys

sys.path.insert(0, "/opt/trn_rl_repo")

import numpy as np

import concourse.bass as bass
import concourse.bacc as bacc
import concourse.mybir as mybir
import concourse.tile as tile
from concourse.bass_utils import run_bass_kernel_spmd
from concourse.masks import make_identity

FP16 = mybir.dt.float16
F32 = mybir.dt.float32
I32 = mybir.dt.int32

S, B, H, E, V = 64, 64, 1024, 512, 32000
NCORES = 8
VC = V // NCORES          # 4000 vocab cols per core
SB = S * B                # 4096
KH = H // 128             # 8 h k-chunks
KE = E // 128             # 4 e k-chunks
NQ = 4                    # output job-pairs per 128-token tile
NT = SB // 128            # 32 token tiles (also output row tiles)
G3 = 3 * H                # 3072 gate cols
GLEAD = 4                 # gi tiles emitted ahead of the step loop

_CACHE = {}


def _build(n_steps=S, with_jobs=True):
    key = ("nc", n_steps, with_jobs)
    if key in _CACHE:
        return _CACHE[key]

    nc = bacc.Bacc("TRN2", target_bir_lowering=False, debug=False)

    def din(name, shape, dt):
        return nc.dram_tensor(name, shape, dt, kind="ExternalInput").ap()

    emb_d = din("emb_t", [V, E], FP16)
    idx_d = din("idx", [128, NT], I32)
    w_rzA_d = din("w_rzA", [128, KH, 1024], FP16)
    w_rzB_d = din("w_rzB", [128, KH, 1024], FP16)
    w_hnA_d = din("w_hnA", [128, KH, 512], FP16)
    w_hnB_d = din("w_hnB", [128, KH, 512], FP16)
    w_iflat_d = din("w_iflat", [128, KE, G3], FP16)
    gi_bias_d = din("gi_bias", [128, G3], FP16)
    bias_nhh_d = din("bias_nhh", [128, 512], F32)
    hT0_d = din("hT0", [128, 4, 128], FP16)
    h0st_d = din("h0st", [128, 512], FP16)
    w_outT_d = din("w_outT", [128, KH, VC], FP16)
    b_out_d = din("b_out_pair", [128, NQ, 1024], FP16)
    out_d = nc.dram_tensor("out", [SB, VC], FP16, kind="ExternalOutput").ap()

    # per-tile DRAM staging for gi (one tensor per 128-token tile so tile
    # dep tracking, whole-tensor granular on DRAM, stays exact)
    gi_d = [nc.dram_tensor(f"gi{j}", [128, G3], FP16).ap() for j in range(NT)]

    with tile.TileContext(nc) as tc:
        with tc.tile_pool(name="const", bufs=1) as pc, \
             tc.tile_pool(name="roll", bufs=1) as pr, \
             tc.tile_pool(name="psum", bufs=1, space="PSUM") as pp:

            # ---- constants in SBUF
            w_rzA = pc.tile([128, KH, 1024], FP16, name="w_rzA")
            w_rzB = pc.tile([128, KH, 1024], FP16, name="w_rzB")
            w_hnA = pc.tile([128, KH, 512], FP16, name="w_hnA")
            w_hnB = pc.tile([128, KH, 512], FP16, name="w_hnB")
            w_iflat = pc.tile([128, KE, G3], FP16, name="w_iflat")
            gi_bias = pc.tile([128, G3], FP16, name="gi_bias")
            bias_nhh = pc.tile([128, 512], F32, name="bias_nhh")
            w_outT = pc.tile([128, KH, VC], FP16, name="w_outT")
            b_out = pc.tile([128, NQ, 1024], FP16, name="b_out")
            idx = pc.tile([128, NT], I32, name="idx")
            ident = pc.tile([128, 128], FP16, name="ident")

            # DMA order: idx first (gates the gathers), then w_iflat (gates
            # the gi precompute that fills the startup window), then the
            # recurrence weights, output consts last.
            nc.sync.dma_start(out=idx[:], in_=idx_d[:])
            make_identity(nc, ident[:])

            hT = pr.tile([128, 4, 128], FP16, name="hT_init", tag="hT", bufs=2)
            h_st = pr.tile([128, 512], FP16, name="hst_init", tag="hst",
                           bufs=2)
            nc.sync.dma_start(out=hT[:], in_=hT0_d[:])
            nc.sync.dma_start(out=h_st[:], in_=h0st_d[:])

            for t, d in [(w_iflat, w_iflat_d), (gi_bias, gi_bias_d),
                         (bias_nhh, bias_nhh_d),
                         (w_rzA, w_rzA_d), (w_rzB, w_rzB_d),
                         (w_hnA, w_hnA_d), (w_hnB, w_hnB_d),
                         (b_out, b_out_d), (w_outT, w_outT_d)]:
                nc.sync.dma_start(out=t[:], in_=d[:])

            # ---- HAM warmup: dummy PE transposes spanning the initial
            # weight-DMA window.  The PE clock gate needs ~3.4us of
            # sustained busy to reach 2.4 GHz and re-throttles after any
            # ~3.4us idle; without these the whole gi/recurrence startup
            # runs at 1.2 GHz.
            psd = pp.tile([128, 128], FP16, name="ps_warm", tag="psT",
                          bufs=1)
            for _ in range(256):
                nc.tensor.matmul(out=psd[:], lhsT=ident[:], rhs=ident[:],
                                 is_transpose=True, start=True, stop=True,
                                 skip_group_check=True)

            # ---- batched gi precompute, one 128-token tile at a time
            def emit_gi(j):
                er = pr.tile([128, E], FP16, name=f"er{j}", tag="er", bufs=2)
                nc.gpsimd.indirect_dma_start(
                    out=er[:], out_offset=None,
                    in_=emb_d[:],
                    in_offset=bass.IndirectOffsetOnAxis(
                        ap=idx[:, j:j + 1], axis=0),
                )
                ps_e = pp.tile([128, 512], FP16, name=f"psE{j}", tag="psT",
                               bufs=1)
                for m in range(KE):
                    nc.tensor.matmul(
                        out=ps_e[:, 128 * m:128 * m + 128],
                        lhsT=er[:, 128 * m:128 * m + 128],
                        rhs=ident[:],
                        is_transpose=True, start=(m == 0), stop=(m == KE - 1),
                        skip_group_check=True)
                eT = pr.tile([128, KE, 128], FP16, name=f"eT{j}", tag="eT",
                             bufs=2)
                nc.vector.tensor_copy(
                    out=eT[:], in_=ps_e[:].rearrange("p (m c) -> p m c", m=KE))
                gi16 = pr.tile([128, G3], FP16, name=f"gi16_{j}", tag="gi16",
                               bufs=2)
                for v in range(6):
                    ps = pp.tile([128, 512], F32, name=f"psg{j}_{v}",
                                 tag="psgi", bufs=2)
                    for k in range(KE):
                        nc.tensor.matmul(
                            out=ps[:], lhsT=eT[:, k, :],
                            rhs=w_iflat[:, k, 512 * v:512 * v + 512],
                            start=(k == 0), stop=(k == KE - 1),
                            skip_group_check=True)
                    nc.vector.tensor_tensor(
                        out=gi16[:, 512 * v:512 * v + 512], in0=ps[:],
                        in1=gi_bias[:, 512 * v:512 * v + 512],
                        op=mybir.AluOpType.add)
                # sync ring (NOT scalar: there it waited ~16us per tile at
                # the ring head, blocking the gate-chain activations queued
                # behind it)
                nc.sync.dma_start(out=gi_d[j][:], in_=gi16[:])

            for j in range(min(GLEAD, NT)):
                emit_gi(j)

            # ---- per-step gi slices in stacked layout, prefetched
            def fetch_gi(s):
                t = pr.tile([128, 3 * 512], FP16, name=f"giS{s}", tag="giS",
                            bufs=3)
                view = gi_d[s // 2][64 * (s % 2):64 * (s % 2) + 64,
                                    :].rearrange(
                    "b (g hf j) -> hf b g j", g=3, hf=2, j=512)
                for hf in (0, 1):
                    # sync ring (free after const loads) — keeps the reads
                    # out of the scalar ring where the gi_d writes queue
                    nc.sync.dma_start(
                        out=t[64 * hf:64 * hf + 64, :].rearrange(
                            "p (g j) -> p g j", g=3, j=512),
                        in_=view[hf])
                return t

            gi_w = {s: fetch_gi(s) for s in range(min(3, n_steps))}

            hs_w = {}     # output-ready hidden tiles: t -> [128, KH, 128] fp16

            jobs = [(t, q) for t in range(n_steps // 2) for q in range(NQ)]
            if not with_jobs:
                jobs = []
            jp = 0  # job pointer

            def emit_job(t, q):
                ps_o = pp.tile([128, 1024], F32, name=f"pso{t}_{q}",
                               tag="pso", bufs=1)
                hst_t = hs_w[t]
                for k in range(KH):
                    lh = hst_t[:, k, :]
                    st, sp = (k == 0), (k == KH - 1)
                    nc.tensor.matmul(
                        out=ps_o[:, 0:500], lhsT=lh,
                        rhs=w_outT[:, k, 1000 * q:1000 * q + 500],
                        start=st, stop=sp, skip_group_check=True)
                    nc.tensor.matmul(
                        out=ps_o[:, 512:1012], lhsT=lh,
                        rhs=w_outT[:, k, 1000 * q + 500:1000 * q + 1000],
                        start=st, stop=sp, skip_group_check=True)
                ob = pr.tile([128, 1024], FP16, name=f"ob{t}_{q}", tag="ob",
                             bufs=2)
                nc.vector.tensor_tensor(
                    out=ob[:, 0:1012], in0=ps_o[:, 0:1012],
                    in1=b_out[:, q, 0:1012], op=mybir.AluOpType.add)
                nc.sync.dma_start(
                    out=out_d[t * 128:(t + 1) * 128,
                              1000 * q:1000 * q + 500],
                    in_=ob[:, 0:500])
                nc.sync.dma_start(
                    out=out_d[t * 128:(t + 1) * 128,
                              1000 * q + 500:1000 * q + 1000],
                    in_=ob[:, 512:1012])

            for s in range(n_steps):
                g, half = s // 2, s % 2

                # ---- recurrence h-part matmuls (lanes A/B = psum halves).
                # Piece order r -> hn -> z: the r/tanh chain is the longest
                # serial path, the z gate is only needed at the end.
                ps_rz = pp.tile([128, 1024], F32, name=f"psrz{s}", tag="psrz",
                                bufs=1)
                ps_hn = pp.tile([128, 512], F32, name=f"pshn{s}", tag="pshn",
                                bufs=1)

                def lhk(k):
                    return hT[:, k % 4, 64 * (k // 4):64 * (k // 4) + 64]

                for k in range(KH):
                    lh, st, sp = lhk(k), (k == 0), (k == KH - 1)
                    nc.tensor.matmul(out=ps_rz[0:64, 0:512], lhsT=lh,
                                     rhs=w_rzA[:, k, 0:512], start=st, stop=sp,
                                     skip_group_check=True)
                    nc.tensor.matmul(out=ps_rz[64:128, 0:512], lhsT=lh,
                                     rhs=w_rzB[:, k, 0:512], start=st, stop=sp,
                                     skip_group_check=True)
                for k in range(KH):
                    lh, st, sp = lhk(k), (k == 0), (k == KH - 1)
                    nc.tensor.matmul(out=ps_hn[0:64, :], lhsT=lh,
                                     rhs=w_hnA[:, k, :],
                                     start=st, stop=sp, skip_group_check=True)
                    nc.tensor.matmul(out=ps_hn[64:128, :], lhsT=lh,
                                     rhs=w_hnB[:, k, :],
                                     start=st, stop=sp, skip_group_check=True)
                for k in range(KH):
                    lh, st, sp = lhk(k), (k == 0), (k == KH - 1)
                    nc.tensor.matmul(out=ps_rz[0:64, 512:1024], lhsT=lh,
                                     rhs=w_rzA[:, k, 512:1024], start=st,
                                     stop=sp, skip_group_check=True)
                    nc.tensor.matmul(out=ps_rz[64:128, 512:1024], lhsT=lh,
                                     rhs=w_rzB[:, k, 512:1024], start=st,
                                     stop=sp, skip_group_check=True)

                # ---- prefetch gi slice for a later step.  Emitted FIRST so
                # it lands on the sync ring ahead of this step's output DMAs
                # (whose data isn't ready yet — head-of-line blocking here
                # starves the recurrence of gi and stalls the PE).
                if s + 3 < n_steps:
                    gi_w[s + 3] = fetch_gi(s + 3)

                # ---- gate chain.  DVE keeps the PSUM-touching ops (Pool
                # cannot read PSUM); gpsimd takes the SBUF-only tail.
                giS = gi_w.pop(s)
                rz = pr.tile([128, 1024], FP16, name=f"rz{s}", tag="rz",
                             bufs=2)
                nc.vector.tensor_tensor(out=ps_rz[:, 0:512],
                                        in0=ps_rz[:, 0:512],
                                        in1=giS[:, 0:512],
                                        op=mybir.AluOpType.add)
                nc.scalar.activation(out=rz[:, 0:512], in_=ps_rz[:, 0:512],
                                     func=mybir.ActivationFunctionType.Sigmoid)
                nc.vector.tensor_tensor(out=ps_hn[:], in0=ps_hn[:],
                                        in1=bias_nhh[:],
                                        op=mybir.AluOpType.add)
                tn = pr.tile([128, 512], FP16, name=f"tn{s}", tag="tn",
                             bufs=2)
                nc.vector.tensor_tensor(out=tn[:], in0=rz[:, 0:512],
                                        in1=ps_hn[:],
                                        op=mybir.AluOpType.mult)
                nc.vector.tensor_tensor(out=ps_rz[:, 512:1024],
                                        in0=ps_rz[:, 512:1024],
                                        in1=giS[:, 512:1024],
                                        op=mybir.AluOpType.add)
                nc.gpsimd.tensor_tensor(out=tn[:], in0=tn[:],
                                        in1=giS[:, 1024:1536],
                                        op=mybir.AluOpType.add)
                n_sb = pr.tile([128, 512], FP16, name=f"n{s}", tag="n",
                               bufs=1)
                nc.scalar.activation(out=n_sb[:], in_=tn[:],
                                     func=mybir.ActivationFunctionType.Tanh)
                nc.scalar.activation(out=rz[:, 512:1024],
                                     in_=ps_rz[:, 512:1024],
                                     func=mybir.ActivationFunctionType.Sigmoid)
                d_sb = pr.tile([128, 512], FP16, name=f"d{s}", tag="d",
                               bufs=1)
                nc.vector.tensor_tensor(out=d_sb[:], in0=h_st[:], in1=n_sb[:],
                                        op=mybir.AluOpType.subtract)
                nc.vector.tensor_tensor(out=d_sb[:], in0=rz[:, 512:1024],
                                        in1=d_sb[:],
                                        op=mybir.AluOpType.mult)
                h_st = pr.tile([128, 512], FP16, name=f"hst{s}", tag="hst",
                               bufs=2)
                nc.vector.tensor_tensor(out=h_st[:], in0=n_sb[:], in1=d_sb[:],
                                        op=mybir.AluOpType.add)

                # ---- PE filler sized to the gate-chain latency window: gi
                # tile (phase 1) or two job-pairs (phase 2) run while the
                # chain computes h'; then the transpose; then the rest of
                # the jobs overlap the NEXT step's chain.
                nj = 2 if s + GLEAD < NT else 3
                njb = 0 if s + GLEAD < NT else 2
                emitted = 0
                if s + GLEAD < NT:
                    emit_gi(s + GLEAD)
                if s >= 3:
                    while (emitted < njb and jp < len(jobs)
                           and 2 * jobs[jp][0] + 2 <= s):
                        emit_job(*jobs[jp])
                        jp += 1
                        emitted += 1

                # ---- transpose h back to lhsT layout (h_st is fp16 now, so
                # the PE reads it directly — no cast).  The hT copy runs on
                # the ACT engine (free after the sigmoids) so it never queues
                # behind epilogue adds on the DVE — it gates the next step's
                # recurrence matmuls.
                ps_T = pp.tile([128, 512], FP16, name=f"psT{s}", tag="psT",
                               bufs=1)
                for m in range(4):
                    nc.tensor.matmul(
                        out=ps_T[:, 128 * m:128 * m + 128],
                        lhsT=h_st[:, 128 * m:128 * m + 128],
                        rhs=ident[:],
                        is_transpose=True, start=(m == 0), stop=(m == 3),
                        skip_group_check=True)
                hT = pr.tile([128, 4, 128], FP16, name=f"hT{s}", tag="hT",
                             bufs=2)
                nc.scalar.copy(
                    out=hT[:], in_=ps_T[:].rearrange("p (m c) -> p m c", m=4))
                if half == 0:
                    hs_w[g] = pr.tile([128, KH, 128], FP16, name=f"hs{g}",
                                      tag="hs", bufs=3)
                nc.vector.tensor_copy(
                    out=hs_w[g][:, :, 64 * half:64 * half + 64],
                    in_=ps_T[:].rearrange("p (m hh b) -> p hh m b", m=4, hh=2))

                # ---- remaining jobs overlap the next step's gate chain
                if s >= 3:
                    while (emitted < nj and jp < len(jobs)
                           and 2 * jobs[jp][0] + 2 <= s):
                        emit_job(*jobs[jp])
                        jp += 1
                        emitted += 1

            # ---- drain remaining output jobs
            while jp < len(jobs):
                emit_job(*jobs[jp])
                jp += 1

    nc.compile()
    _CACHE[key] = nc
    return nc


def _prep_in_maps(x, hidden, emb, w_ih, w_hh, b_ih, b_hh, w_out, b_out):
    f16, f32 = np.float16, np.float32

    toks = np.concatenate([np.full((1, B), 2, dtype=np.int64),
                           np.asarray(x)[:-1].astype(np.int64)], axis=0)
    t_flat = toks.reshape(SB).astype(np.int32)
    idx = np.ascontiguousarray(t_flat.reshape(NT, 128).T)        # [128, 32]

    emb_t = np.asarray(emb, dtype=f32).copy()
    emb_t[0] = 0.0
    emb_t = np.maximum(emb_t, 0.0).astype(f16)                    # relu folded

    w_hh = np.asarray(w_hh, dtype=f32)
    w_ih = np.asarray(w_ih, dtype=f32)
    Wr, Wz, Wn = w_hh[0:H], w_hh[H:2 * H], w_hh[2 * H:3 * H]

    def kview(m, kc):  # [rows, K] -> [128, kc, rows] fp16 (K on partitions)
        return np.ascontiguousarray(
            m.T.reshape(kc, 128, m.shape[0]).transpose(1, 0, 2)).astype(f16)

    w_rzA = kview(np.concatenate([Wr[0:512], Wz[0:512]], 0), KH)
    w_rzB = kview(np.concatenate([Wr[512:1024], Wz[512:1024]], 0), KH)
    w_hnA = kview(Wn[0:512], KH)
    w_hnB = kview(Wn[512:1024], KH)
    w_iflat = kview(w_ih, KE)                                     # [128,4,3072]

    b_ih = np.asarray(b_ih, dtype=f32)
    b_hh = np.asarray(b_hh, dtype=f32)
    gb = np.concatenate([b_ih[0:2 * H] + b_hh[0:2 * H], b_ih[2 * H:3 * H]])
    gi_bias = np.ascontiguousarray(
        np.broadcast_to(gb, (128, G3))).astype(f16)
    bias_nhh = np.empty((128, 512), f32)
    for hp in (0, 1):
        r = slice(64 * hp, 64 * hp + 64)
        bias_nhh[r] = b_hh[2 * H:3 * H][512 * hp:512 * hp + 512][None, :]

    h0 = np.asarray(hidden, dtype=f32)[0]                         # [B, H]
    # hT0[p, m, 64*hh + b] = h0[b, 128*(m + 4*hh) + p]
    hT0 = np.ascontiguousarray(
        h0.T.reshape(2, 4, 128, B).transpose(2, 1, 0, 3).reshape(128, 4, 128)
    ).astype(f16)
    h0st = np.concatenate([h0[:, 0:512], h0[:, 512:1024]], axis=0).astype(f16)

    w_out = np.asarray(w_out, dtype=f32)
    b_out = np.asarray(b_out, dtype=f32)

    shared = dict(
        emb_t=emb_t, idx=idx,
        w_rzA=w_rzA, w_rzB=w_rzB, w_hnA=w_hnA, w_hnB=w_hnB,
        w_iflat=w_iflat, gi_bias=gi_bias, bias_nhh=bias_nhh,
        hT0=hT0, h0st=h0st,
    )
    in_maps = []
    for c in range(NCORES):
        sl = slice(c * VC, (c + 1) * VC)
        w_outT = np.ascontiguousarray(
            w_out[sl].T.reshape(KH, 128, VC).transpose(1, 0, 2)).astype(f16)
        bo = b_out[sl]
        b_out_pair = np.zeros((128, NQ, 1024), f16)
        for q in range(NQ):
            b_out_pair[:, q, 0:500] = bo[1000 * q:1000 * q + 500][None]
            b_out_pair[:, q, 512:1012] = bo[1000 * q + 500:1000 * q + 1000][None]
        in_maps.append(dict(shared, w_outT=w_outT, b_out_pair=b_out_pair))
    return in_maps


def _assemble(results):
    full = np.concatenate(
        [r["out"].reshape(S, B, VC) for r in results], axis=2)   # (S, B, V)
    return np.ascontiguousarray(full.transpose(1, 0, 2)[None]).astype(np.float32)


def _run(trace=False, tmpdir=None, **inputs):
    nc = _build()
    in_maps = _prep_in_maps(**inputs)
    res = run_bass_kernel_spmd(nc, in_maps, list(range(NCORES)),
                               trace=trace, tmpdir=tmpdir)
    return _assemble(res.results), res


def kernel(**inputs) -> np.ndarray:
    out, _ = _run(**inputs)
    return out


if __name__ == "__main__":
    rng = np.random.default_rng(0)
    ins = dict(
        x=rng.integers(0, V, (S, B)).astype(np.int32),
        hidden=rng.standard_normal((1, B, H)).astype(np.float32),
        emb=rng.standard_normal((V, E)).astype(np.float32),
        w_ih=rng.uniform(-1 / 32, 1 / 32, (3 * H, E)).astype(np.float32),
        w_hh=rng.uniform(-1 / 32, 1 / 32, (3 * H, H)).astype(np.float32),
        b_ih=rng.uniform(-1 / 32, 1 / 32, (3 * H,)).astype(np.float32),
        b_hh=rng.uniform(-1 / 32, 1 / 32, (3 * H,)).astype(np.float32),
        w_out=rng.uniform(-1 / 32, 1 / 32, (V, H)).astype(np.float32),
        b_out=rng.uniform(-1 / 32, 1 / 32, (V,)).astype(np.float32),
    )
    out = kernel(**ins)
    print("out", out.shape, out.dtype, float(np.abs(out).max()))
